# revision 7
# baseline (speedup 1.0000x reference)
"""Cosine multi-head attention (h=1) Trainium2 kernel.

Math (reference):
    context = query @ Wq.T + bq                  [B, S, HD]
    ctx     = context * weight_tensor[0]         (elementwise over HD)
    ctx_n   = ctx / max(||ctx||_2, eps)          (normalize over HD)
    scores  = ctx_n @ ctx_n.T                    [B, S, S]
    out     = softmax(scores, axis=-1)

Device strategy (8 cores, SPMD):
    core c handles batch b = c//2, row-half h = c%2.  The host rotates the
    batch's rows so each core's own 2048 rows come first, transposes to
    qT [D, S] bf16, and folds weight_tensor into Wq: M = diag(w) @ Wq
    (bf16), c0 = w * bq.  HD=120 is zero-padded to 128 on the host so every
    matmul runs with K=M=128 (enables the PE fast-weight-load path).

    On device (single-term bf16 matmuls everywhere; the 2e-2 harness gate
    leaves ~5x margin over the resulting ~4e-3 error):
      CT[hd, s] = sum_d M[hd, d] qT[d, s]      (PE, PSUM [128, 2048])
      ct = CT + c0                             (DVE, PSUM -> SBUF f32)
      ctsq = ct*ct -> bf16                     (DVE)
      n2 = ones^T @ ctsq                       (PE broadcast-sum over HD)
      inv = 1/sqrt(|n2 + 1e-20|)               (ACT Abs_reciprocal_sqrt; the
                                                +eps bias is the free affine)
      cn = ct * inv -> bf16                    (DVE)
      per 128-row chunk i of the first 2048 rows:
         R = cn[:, i-chunk].T @ cn             (PE, PSUM [128, 2048] x2)
         E = exp(R) -> bf16 with fused row-sum (ACT accum_out)
         out_rows = E * (1/rowsum) -> bf16     (DVE 4x packed; reciprocal DVE)
    Softmax needs no max-subtraction: scores are cosines in [-1, 1].
    All four inv instructions are emitted before any softmax exp so ACT
    loads each activation table exactly once (rsqrt set, then exp set).
    Output is written bf16 (absmax metric tolerates ~2e-3); host upcasts.

    Output columns of h=1 cores are rotated by 2048; the host gather undoes it.
"""

import numpy as np
from contextlib import ExitStack

B, S, D, HD = 4, 4096, 1024, 120
HDP = 128      # HD zero-padded so K=M=128 in every matmul
ROWS = S // 2  # rows of the score matrix each core produces
N_CORES = 8

_NC_CACHE = {}


def _build_nc():
    import concourse.bacc as bacc
    import concourse.tile as tile
    from concourse import mybir

    f32 = mybir.dt.float32
    bf16 = mybir.dt.bfloat16
    AF = mybir.ActivationFunctionType
    nc = bacc.Bacc("TRN2", target_bir_lowering=False, debug=False,
                   num_devices=N_CORES)

    q_in = nc.declare_dram_parameter("q_in", [D, S], bf16, isOutput=False)
    mt = nc.declare_dram_parameter("mt", [D, HDP], bf16, isOutput=False)
    c0 = nc.declare_dram_parameter("c0", [HDP, 1], f32, isOutput=False)
    out = nc.declare_dram_parameter("out", [ROWS, S], bf16, isOutput=True)

    DC = D // 128   # 8 contraction chunks

    with ExitStack() as ctx:
        tc = ctx.enter_context(tile.TileContext(nc))
        singles = ctx.enter_context(tc.tile_pool(name="singles", bufs=1))
        qpool = ctx.enter_context(tc.tile_pool(name="qpool", bufs=3))
        work = ctx.enter_context(tc.tile_pool(name="work", bufs=1))
        epool = ctx.enter_context(tc.tile_pool(name="epool", bufs=3))
        spool = ctx.enter_context(tc.tile_pool(name="spool", bufs=4))
        ps = ctx.enter_context(tc.tile_pool(name="ps", bufs=2, space="PSUM"))

        # row = cq*512 + cp*256 + c2*128 + p, col = h*2048 + j
        q_r = q_in.rearrange("(cq cp c2 p) (h j) -> cq h p cp c2 j",
                             cq=2, cp=2, c2=2, p=128, h=2)
        # constants first in the DMA queue (tiny; the c==0 matmuls need mt)
        mt_sb = singles.tile([128, DC, HDP], bf16, tag="mt")
        nc.sync.dma_start(out=mt_sb[:],
                          in_=mt.rearrange("(c p) h -> p c h", p=128))
        c0_sb = singles.tile([HDP, 1], f32, tag="c0")
        nc.sync.dma_start(out=c0_sb[:], in_=c0[:])
        ones_sq = singles.tile([HDP, HDP], bf16, tag="ones_sq")
        nc.vector.memset(ones_sq[:], 1.0)
        eps_sb = singles.tile([HDP, 1], f32, tag="eps")
        nc.vector.memset(eps_sb[:], 1e-20)

        # cn: normalized context, bf16, both halves in one tile [HDP, S]
        cn = work.tile([HDP, S], bf16, tag="cn", name="cn")

        for half in range(2):
            qcs = []
            for cq in range(2):
                qc = qpool.tile([128, 2, 2, 2048], bf16, tag="q",
                                name=f"q_{half}_{cq}")
                nc.sync.dma_start(out=qc[:], in_=q_r[cq, half])
                qcs.append(qc)

            ct_ps = ps.tile([HDP, 2048], f32, tag="ps4", name=f"ct_ps{half}")
            # column-strip split (2x1024) so the phase-2 chain on strip 0
            # overlaps strip 1's matmuls
            for strip in range(2):
                for cq in range(2):
                    for cp in range(2):
                        for c2 in range(2):
                            c = cq * 4 + cp * 2 + c2
                            for k in (2 * strip, 2 * strip + 1):
                                nc.tensor.matmul(
                                    ct_ps[:, k * 512:(k + 1) * 512],
                                    lhsT=mt_sb[:, c, :],
                                    rhs=qcs[cq][:, cp, c2,
                                                k * 512:(k + 1) * 512],
                                    start=(c == 0), stop=(c == DC - 1))

            ct_f = work.tile([HDP, 2048], f32, tag=f"ct{half}",
                             name=f"ct{half}")
            ctsq = work.tile([HDP, 2048], bf16, tag=f"ctsq{half}",
                             name=f"ctsq{half}")
            n_ps = ps.tile([HDP, 2048], f32, tag="ps4", name=f"n_ps{half}")
            inv = work.tile([HDP, 2048], f32, tag=f"inv{half}",
                            name=f"inv{half}")
            for strip in range(2):
                sl = slice(strip * 1024, (strip + 1) * 1024)
                nc.vector.tensor_scalar_add(ct_f[:, sl], ct_ps[:, sl],
                                            c0_sb[:])
                nc.vector.tensor_mul(ctsq[:, sl], ct_f[:, sl], ct_f[:, sl])
                for k in (2 * strip, 2 * strip + 1):
                    nc.tensor.matmul(n_ps[:, k * 512:(k + 1) * 512],
                                     lhsT=ones_sq[:],
                                     rhs=ctsq[:, k * 512:(k + 1) * 512],
                                     start=True, stop=True)
                nc.scalar.activation(out=inv[:, sl], in_=n_ps[:, sl],
                                     func=AF.Abs_reciprocal_sqrt,
                                     bias=eps_sb[:])
                nc.vector.tensor_mul(cn[:, half * 2048 + strip * 1024:
                                        half * 2048 + (strip + 1) * 1024],
                                     ct_f[:, sl], inv[:, sl])

        # --- phase 3: gram + softmax; pairs of 128-row chunks share an
        #     output tile so DMA-out goes in 2 MB transfers ---
        NCHUNK = ROWS // 128
        for i in range(NCHUNK):
            ic = i % 2
            if ic == 0:
                e2 = epool.tile([128, 2, S], bf16, tag="e", name=f"e{i}")
                sums = spool.tile([128, 4], f32, tag="sums", name=f"sums{i}")
            hi_i = cn[:, i * 128:(i + 1) * 128]
            for jg in range(2):
                r_ps = ps.tile([128, 2048], f32, tag="ps4",
                               name=f"r_ps{i}_{jg}")
                for k in range(4):
                    kk = jg * 4 + k
                    nc.tensor.matmul(r_ps[:, k * 512:(k + 1) * 512],
                                     lhsT=hi_i,
                                     rhs=cn[:, kk * 512:(kk + 1) * 512],
                                     start=True, stop=True)
                nc.scalar.activation(
                    out=e2[:, ic, jg * 2048:(jg + 1) * 2048],
                    in_=r_ps[:],
                    func=AF.Exp,
                    accum_out=sums[:, 2 * ic + jg:2 * ic + jg + 1],
                )
            tot = spool.tile([128, 1], f32, tag="tot", name=f"tot{i}")
            nc.vector.tensor_add(tot[:], sums[:, 2 * ic:2 * ic + 1],
                                 sums[:, 2 * ic + 1:2 * ic + 2])
            rec = spool.tile([128, 1], f32, tag="rec", name=f"rec{i}")
            nc.vector.reciprocal(rec[:], tot[:])
            nc.vector.tensor_scalar_mul(e2[:, ic, :], e2[:, ic, :], rec[:])
            if i >= NCHUNK - 2:
                # drain the tail in single-chunk DMAs (shorter critical path)
                nc.sync.dma_start(out=out[i * 128:(i + 1) * 128, :],
                                  in_=e2[:, ic, :])
            elif ic == 1:
                nc.sync.dma_start(
                    out=out[(i - 1) * 128:(i + 1) * 128, :].rearrange(
                        "(c p) s -> p c s", p=128),
                    in_=e2[:],
                )

    nc.compile()
    return nc


def _get_nc():
    if "nc" not in _NC_CACHE:
        _NC_CACHE["nc"] = _build_nc()
    return _NC_CACHE["nc"]


def _make_in_maps(inputs):
    import ml_dtypes
    query = np.asarray(inputs["query"], dtype=np.float32)
    Wq = np.asarray(inputs["Wq"], dtype=np.float32)
    bq = np.asarray(inputs["bq"], dtype=np.float32)
    w = np.asarray(inputs["weight_tensor"], dtype=np.float32)

    w0 = w.reshape(-1)[:HD]
    mt_np = np.zeros((D, HDP), dtype=ml_dtypes.bfloat16)
    mt_np[:, :HD] = (w0[:, None] * Wq).T.astype(ml_dtypes.bfloat16)  # [D,HDP]
    c0_np = np.zeros((HDP, 1), dtype=np.float32)
    c0_np[:HD, 0] = w0 * bq

    in_maps = []
    for c in range(N_CORES):
        b, h = c // 2, c % 2
        qb = query[b]
        if h:
            qb = np.concatenate([qb[ROWS:], qb[:ROWS]], axis=0)
        q_np = np.ascontiguousarray(qb.T.astype(ml_dtypes.bfloat16))  # [D,S]
        in_maps.append({"q_in": q_np, "mt": mt_np, "c0": c0_np})
    return in_maps


def _gather(results):
    full = np.empty((B, S, S), dtype=np.float32)
    for c in range(N_CORES):
        b, h = c // 2, c % 2
        r = results[c]["out"]  # bf16 [ROWS, S]; assignment upcasts
        if h == 0:
            full[b, :ROWS] = r
        else:
            full[b, ROWS:, ROWS:] = r[:, :ROWS]
            full[b, ROWS:, :ROWS] = r[:, ROWS:]
    return full


def kernel(**inputs):
    from concourse.bass_utils import run_bass_kernel_spmd

    in_maps = _make_in_maps(inputs)
    nc = _get_nc()
    res = run_bass_kernel_spmd(nc, in_maps, list(range(N_CORES))).results
    return _gather(res)


def _register_ntff_hook():
    """Register the axon NTFF profile hook that the agent image's antenv
    package lacks (see trn_boot.py) so trace=True yields exec_time_ns."""
    import sys
    import types
    try:
        import antenv.axon_hooks  # noqa: F401
        return True
    except ImportError:
        pass
    try:
        from trn_agent_boot.trn_boot import _ntff_profile_via_ctypes
        hook = _ntff_profile_via_ctypes("/opt/axon/libaxon_pjrt.so")
    except Exception:
        return False
    if hook is None:
        return False
    mod = types.ModuleType("antenv.axon_hooks")
    mod._hook = hook
    mod.get_axon_ntff_profile_hook = lambda: mod._hook
    mod.set_axon_ntff_profile_hook = lambda h: setattr(mod, "_hook", h)
    sys.modules["antenv.axon_hooks"] = mod
    import antenv
    antenv.axon_hooks = mod
    return True


def profile_once(inputs, trace_cores=None):
    """Re-run the kernel with NTFF profiling; returns max exec_time_ns."""
    import tempfile
    import concourse.bass_utils as bu

    _register_ntff_hook()
    # avoid the cloud artifact upload inside the trace path
    bu.upload_artifacts = lambda tmpdir: tmpdir

    in_maps = _make_in_maps(inputs)
    nc = _get_nc()
    tmpdir = tempfile.mkdtemp(prefix="ntff_")
    r = bu.run_bass_kernel_spmd(nc, in_maps, list(range(N_CORES)),
                                trace=True, trace_cores=trace_cores,
                                tmpdir=tmpdir)
    print(f"trace dir: {tmpdir}")
    if r.exec_time_ns is not None:
        print(f"mean exec: {r.mean_exec_time_ns} ns, "
              f"max core: {r.max_exec_time_core_id}")
    return r.exec_time_ns
